# revision 41
# baseline (speedup 1.0000x reference)
"""Trainium2 Bass kernel for nn_Attention_33200097198117.

B=16, N=1025, C=768, H=12 RoPE attention. Data-parallel over batch:
each of the 8 NeuronCores computes 2 batches with the full weights; the
full output is the concatenation over cores (no collectives needed).

kernel(**inputs) -> np.ndarray: builds the Bass/Tile program (cached),
shards inputs, runs on cores 0-7 via bass_utils.run_bass_kernel_spmd,
and concatenates the per-core outputs.
"""

import numpy as np

# ---------------------------------------------------------------------------
# Toolchain compatibility: this container's walrus accepts at most ONE sync
# wait entry per instruction, while Tile's scheduler attaches several (and
# its kernel-tail drain collects one per outstanding semaphore). Patch the
# tail drain and post-process the module to split multi-wait instructions.
# ---------------------------------------------------------------------------
import concourse.tile as tile
from bass_rust import ScopedClock


def _drain_and_barrier(self, tick_clock, wait_clock):
    drain_inst = self.nc.sync.drain()
    wait_clock.add_sem_waits(drain_inst.ins, ScopedClock({None: tick_clock.global_clock}))
    si = drain_inst.ins.sync_info
    waits = list(si.on_wait) if si is not None else []
    if len(waits) > 1:
        si.on_wait = [waits[0]]
        assert self.sems is not None
        allocated = dict(self.sems.allocated())
        by_name = {}
        for v in allocated.values():
            by_name[getattr(v, "name", None)] = v
        for w in waits[1:]:
            sem = by_name.get(w.ant_name) or allocated.get(w.ant_name)
            assert sem is not None, f"sem {w.ant_name} not found"
            nop = self.nc.sync.nop()
            assert w.wait_mode in ("sem-ge-imm", "sem-ge"), w.wait_mode
            nop.wait_op(sem, w.wait_value, "sem-ge")

    self.nc.all_engine_barrier()
    assert self.sems is not None
    popped = self.nc._tile_sem_poison_stack.pop()
    assert popped is self._sem_poison
    self.nc.clear_and_free_semaphores(list(self.sems.allocated().values()))
    self.nc.all_engine_barrier()


tile.TileContext._drain_and_barrier = _drain_and_barrier


def split_multi_waits(nc):
    """Hoist extra sync waits onto cloned NoOps before each instruction."""
    import copy
    import bass_rust

    template = None
    for f in nc.m.functions:
        for b in f.blocks:
            for inst in b.instructions:
                if type(inst).__name__ == "InstNoOp":
                    template = inst
                    break
            if template is not None:
                break
    assert template is not None, "need one InstNoOp in module as clone template"

    for f in nc.m.functions:
        for b in f.blocks:
            changed = False
            out = []
            for inst in b.instructions:
                si = inst.sync_info
                waits = list(si.on_wait) if si is not None else []
                if len(waits) > 1:
                    changed = True
                    for i, w in enumerate(waits[:-1]):
                        n = copy.copy(template)
                        n.name = f"{inst.name}-wsplit{i}"
                        n.engine = inst.engine
                        n.sync_info = bass_rust.SyncInfo(on_wait=[w], on_update=[])
                        out.append(n)
                    si.on_wait = [waits[-1]]
                out.append(inst)
            if changed:
                b.instructions = out


_DOC = """Bass/Tile kernel for nn_Attention (B=16, N=1025, C=768, H=12 RoPE attention).

Sharding: data-parallel over batch. Each of 8 cores processes 2 batches with
full weights; no collectives.

v3: bf16 matmuls, weights resident in SBUF, and software-pipelined emission:
the attention inner loops (ACT-exp-bound) are interleaved chunk-by-chunk with
the NEXT group's QKV projection / transpose / output-projection matmuls so
the PE never idles on softmax waits. Double-buffered xT/qkT/v_aug/attn_outT;
PSUM split into dedicated single-buffer regions per stream (o_ps / S-tile /
qkv accumulator) + 2 shared small banks.

Per-(batch,group) math is identical to v2 (see emit_* functions).
"""

from contextlib import ExitStack

import concourse.bass as bass
import concourse.mybir as mybir
import concourse.tile as tile
from concourse.masks import make_identity

F32 = mybir.dt.float32
F32R = mybir.dt.float32r
BF = mybir.dt.bfloat16
AF = mybir.ActivationFunctionType

B_PER_CORE = 2
N = 1025
C = 768
H = 12
DH = 64
NT = 8          # full 128-token tiles
NPAD = 1152     # qkT free-dim allocation (1024 + 128 zero pad incl. col 1024)
SCALE = DH ** -0.5


def build_rot_matrix(nc, rot):
    """lhsT for rotate_half: out = rot.T @ qT gives rot(q) rows."""
    nc.gpsimd.memset(rot, 0.0)
    for blk in range(2):
        b0 = 64 * blk
        nc.gpsimd.affine_select(
            out=rot[b0:b0 + 32, :],
            in_=rot[b0:b0 + 32, :],
            compare_op=mybir.AluOpType.not_equal,
            fill=1.0,
            base=b0 + 32,
            pattern=[[-1, 128]],
            channel_multiplier=1,
        )
        nc.gpsimd.affine_select(
            out=rot[b0 + 32:b0 + 64, :],
            in_=rot[b0 + 32:b0 + 64, :],
            compare_op=mybir.AluOpType.not_equal,
            fill=-1.0,
            base=b0,
            pattern=[[-1, 128]],
            channel_multiplier=1,
        )


def build_kernel():
    nc = bass.Bass("TRN2", target_bir_lowering=False, debug=False, num_devices=8)

    x = nc.dram_tensor("x", [B_PER_CORE, N, C], F32, kind="ExternalInput").ap()
    sin = nc.dram_tensor("sin", [N - 1, DH], F32, kind="ExternalInput").ap()
    cos = nc.dram_tensor("cos", [N - 1, DH], F32, kind="ExternalInput").ap()
    w_qkv = nc.dram_tensor("w_qkv", [C, 3 * C], F32, kind="ExternalInput").ap()
    w_proj = nc.dram_tensor("w_proj", [C, C], F32, kind="ExternalInput").ap()
    b_proj = nc.dram_tensor("b_proj", [C], F32, kind="ExternalInput").ap()
    y = nc.dram_tensor("y", [B_PER_CORE, N, C], F32, kind="ExternalOutput").ap()

    with tile.TileContext(nc) as tc, ExitStack() as ctx:
        nc.sync.nop(nofuse=True)  # clone template for split_multi_waits
        const = ctx.enter_context(tc.tile_pool(name="const", bufs=1))
        big = ctx.enter_context(tc.tile_pool(name="bigbuf", bufs=1))
        xn_pool = ctx.enter_context(tc.tile_pool(name="xnat", bufs=2))
        pt_pool = ctx.enter_context(tc.tile_pool(name="pt", bufs=3))
        tmp_pool = ctx.enter_context(tc.tile_pool(name="tmp", bufs=2))
        y_pool = ctx.enter_context(tc.tile_pool(name="ystage", bufs=2))
        nrm_pool = ctx.enter_context(tc.tile_pool(name="nrm", bufs=1))
        psum = ctx.enter_context(tc.tile_pool(name="psum", bufs=1, space="PSUM"))
        psum_s = ctx.enter_context(tc.tile_pool(name="psum_s", bufs=2, space="PSUM"))

        # PSUM budget (8 banks): o_ps 2 + S-ping-pong 4 + filler-smalls 2.
        # Attention owns "ops"/"sc"; the interleaved filler stream owns
        # "small" exclusively (so fillers may hold smalls across yields).
        def po():
            return psum.tile([128, 1024], F32, tag="ops", name="po")

        def ps():
            return psum.tile([128, 1024], F32, tag="sc", bufs=2, name="ps")

        def psmall():
            return psum_s.tile([128, 512], F32, tag="small", name="psmall")

        # ---------------- constants ----------------
        identf = const.tile([128, 128], F32, tag="identf")
        make_identity(nc, identf[:])
        ident = const.tile([128, 128], F32R, tag="ident")
        nc.vector.tensor_copy(out=ident[:], in_=identf[:])
        rotf = const.tile([128, 128], F32, tag="rotf")
        build_rot_matrix(nc, rotf[:])
        rot = const.tile([128, 128], BF, tag="rot")
        nc.vector.tensor_copy(out=rot[:], in_=rotf[:])
        onesf = const.tile([128, 1], F32, tag="onesf")
        nc.vector.memset(onesf[:], 1.0)
        zerof = const.tile([128, 1], F32, tag="zerof")
        nc.vector.memset(zerof[:], 0.0)
        ones64 = const.tile([1, 64], F32R, tag="ones64")
        nc.vector.tensor_copy(out=ones64[:], in_=onesf[0:1, 0:1].to_broadcast([1, 64]))
        ones64b = const.tile([1, 64], BF, tag="ones64b")
        nc.vector.tensor_copy(out=ones64b[:], in_=onesf[0:1, 0:1].to_broadcast([1, 64]))

        bias_bc = const.tile([128, C], F32, tag="bias")
        nc.sync.dma_start(bias_bc[0:1, :], b_proj[None, :])
        p = 1
        while p < 128:
            nc.sync.dma_start(bias_bc[p:2 * p, :], bias_bc[0:p, :])
            p *= 2

        # sinT/cosT: [128, N] bf16 coeff col t = (sin,cos) for token t.
        sinT = const.tile([128, N], BF, tag="sinT")
        cosT = const.tile([128, N], BF, tag="cosT")
        nc.vector.memset(sinT[:, 0:1], 0.0)
        nc.vector.memset(cosT[:, 0:1], 1.0)
        sin_nat = sin.rearrange("(o p) d -> p o d", p=128)
        cos_nat = cos.rearrange("(o p) d -> p o d", p=128)
        for src_nat, dstT in ((sin_nat, sinT), (cos_nat, cosT)):
            for t in range(NT):
                nat = tmp_pool.tile([128, DH], F32R, tag="scnat")
                nc.sync.dma_start(nat[:], src_nat[:, t, :].bitcast(F32R))
                pt = psmall()
                nc.tensor.transpose(pt[0:DH, 0:128].bitcast(F32R), nat[:], ident[:])
                nc.vector.tensor_copy(
                    out=dstT[0:DH, 1 + 128 * t:1 + 128 * (t + 1)],
                    in_=pt[0:DH, 0:128],
                )
        nc.sync.dma_start(sinT[64:128, :], sinT[0:64, :])
        nc.sync.dma_start(cosT[64:128, :], cosT[0:64, :])

        # ---------------- resident weights (bf16, loaded once) ----------------
        wq = big.tile([128, 6, 18, 128], BF, tag="wq")
        wdram = w_qkv.rearrange("(k p) (t q) -> p k t q", p=128, q=128)
        for k in range(6):
            for h0 in (0, 9):
                wstage = tmp_pool.tile([128, 9, 128], F32, tag="wstage")
                nc.sync.dma_start(wstage[:], wdram[:, k, h0:h0 + 9, :])
                nc.vector.tensor_copy(out=wq[:, k, h0:h0 + 9, :], in_=wstage[:])
        wproj = big.tile([128, 6, C], BF, tag="wproj")
        for ct in range(6):
            pstage = tmp_pool.tile([128, 9, 128], F32, tag="wstage", name="pstage")
            nc.sync.dma_start(
                pstage[:, 0:6, :],
                w_proj[128 * ct:128 * (ct + 1), :].rearrange("p (a b) -> p a b", b=128))
            nc.vector.tensor_copy(
                out=wproj[:, ct, :],
                in_=pstage[:, 0:6, :].rearrange("p a b -> p (a b)"))

        # ---------------- emission generators ----------------
        def gen_xT(b, xT):
            """Transpose x[b] into xT (bf16). Yields between PSUM chunks."""
            for t in range(NT):
                xnat = xn_pool.tile([128, C], F32R, tag="xnat")
                nc.sync.dma_start(xnat[:], x[b, 128 * t:128 * (t + 1), :].bitcast(F32R))
                for kk in range(0, 6, 4):
                    kw = min(4, 6 - kk)
                    pt = psmall()
                    for j in range(kw):
                        nc.tensor.transpose(
                            pt[:, 128 * j:128 * (j + 1)].bitcast(F32R),
                            xnat[:, 128 * (kk + j):128 * (kk + j + 1)],
                            ident[:],
                        )
                    nc.vector.tensor_copy(
                        out=xT[:, kk:kk + kw, 128 * t:128 * (t + 1)],
                        in_=pt[:, 0:128 * kw].rearrange("p (a b) -> p a b", a=kw),
                    )
                    yield
            xtail = nrm_pool.tile([128, 6, 1], F32, tag=f"xtail{b}")
            with nc.allow_non_contiguous_dma(reason="single tail token scatter"):
                nc.sync.dma_start(
                    xtail[:],
                    x[b, 1024, :].rearrange("(k p a) -> p k a", p=128, a=1),
                )
            nc.vector.tensor_copy(out=xT[:, :, 1024:1025], in_=xtail[:])
            yield

        def gen_qkv(b, g, xT, qkT, v_aug):
            """QKV projection + rope for head group g. Yields ~every 0.5-1us
            of PE work so attention can interleave."""
            nc.vector.tensor_copy(
                out=v_aug[:, :, :, DH:DH + 1].rearrange("p a b c -> p (a b) c"),
                in_=onesf[:, 0:1].to_broadcast([128, (NT + 1) * 6, 1]))

            def wqk(k, m):
                t = 3 * g + m if m < 3 else 6 + 3 * g + (m - 3)
                return wq[:, k, t, :]

            for m in range(6):
                qp_a = psmall()
                qp_b = psmall()
                for k in range(6):
                    nc.tensor.matmul(
                        qp_a[:, 0:512],
                        lhsT=wqk(k, m),
                        rhs=xT[:, k, 0:512],
                        start=(k == 0), stop=(k == 5),
                    )
                    nc.tensor.matmul(
                        qp_b[:, 0:512],
                        lhsT=wqk(k, m),
                        rhs=xT[:, k, 512:1024],
                        start=(k == 0), stop=(k == 5),
                    )
                    yield
                raw = tmp_pool.tile([128, 1024], BF, tag="qkraw")
                nc.vector.tensor_copy(out=raw[:, 0:512], in_=qp_a[:, 0:512])
                nc.vector.tensor_copy(out=raw[:, 512:1024], in_=qp_b[:, 0:512])
                rp_a = psmall()
                rp_b = psmall()
                nc.tensor.matmul(rp_a[:, 0:512], lhsT=rot[:],
                                 rhs=raw[:, 0:512], start=True, stop=True)
                nc.tensor.matmul(rp_b[:, 0:512], lhsT=rot[:],
                                 rhs=raw[:, 512:1024], start=True, stop=True)
                yield
                t1 = tmp_pool.tile([128, 1024], BF, tag="ropet1")
                nc.vector.tensor_tensor(
                    t1[:, 0:512], rp_a[:, 0:512], sinT[:, 0:512],
                    mybir.AluOpType.mult)
                nc.vector.tensor_tensor(
                    t1[:, 512:1024], rp_b[:, 0:512], sinT[:, 512:1024],
                    mybir.AluOpType.mult)
                nc.vector.tensor_tensor(
                    raw[:], raw[:], cosT[:, 0:1024], mybir.AluOpType.mult)
                nc.gpsimd.tensor_tensor(
                    qkT[:, m, 0:1024], t1[:], raw[:], mybir.AluOpType.add)
                yield

            for t in range(NT):
                vp = psmall()
                for k in range(6):
                    nc.tensor.matmul(
                        vp[:, 0:384],
                        lhsT=xT[:, k, 128 * t:128 * (t + 1)],
                        rhs=wq[:, k, 12 + 3 * g:12 + 3 * g + 3, :].rearrange(
                            "p a b -> p (a b)"),
                        start=(k == 0), stop=(k == 5),
                    )
                nc.vector.tensor_copy(
                    out=v_aug[:, t, :, 0:DH],
                    in_=vp[:, 0:384].rearrange("p (a b) -> p a b", a=6),
                )
                yield

            # tail token: row-form qkv. v first (1 psmall), then q/k as a
            # psmall pair — never more than 2 live smalls.
            tail_v = psmall()
            for k in range(6):
                nc.tensor.matmul(
                    tail_v[0:1, 0:384],
                    lhsT=xT[:, k, 1024:1025],
                    rhs=wq[:, k, 12 + 3 * g:12 + 3 * g + 3, :].rearrange(
                        "p a b -> p (a b)"),
                    start=(k == 0), stop=(k == 5),
                )
            nc.vector.tensor_copy(
                out=v_aug[:, NT, :, :],
                in_=zerof[:, 0:1].to_broadcast([128, 6, DH + 1]))
            nc.vector.tensor_copy(
                out=v_aug[0:1, NT, :, 0:DH],
                in_=tail_v[0:1, 0:384].rearrange("p (a b) -> p a b", a=6),
            )
            nc.vector.tensor_copy(
                out=v_aug[0:1, NT, :, DH:DH + 1],
                in_=onesf[0:1, 0:1].to_broadcast([1, 6, 1]))
            yield

            tail_qa = psmall()
            tail_qb = psmall()
            for k in range(6):
                nc.tensor.matmul(
                    tail_qa[0:1, 0:384],
                    lhsT=xT[:, k, 1024:1025],
                    rhs=wq[:, k, 3 * g:3 * g + 3, :].rearrange("p a b -> p (a b)"),
                    start=(k == 0), stop=(k == 5),
                )
                nc.tensor.matmul(
                    tail_qb[0:1, 0:384],
                    lhsT=xT[:, k, 1024:1025],
                    rhs=wq[:, k, 6 + 3 * g:6 + 3 * g + 3, :].rearrange(
                        "p a b -> p (a b)"),
                    start=(k == 0), stop=(k == 5),
                )

            tail_qk_sb = nrm_pool.tile([1, 768], F32R, tag=f"tailqksb{g}")
            nc.vector.tensor_copy(out=tail_qk_sb[0:1, 0:384],
                                  in_=tail_qa[0:1, 0:384])
            nc.vector.tensor_copy(out=tail_qk_sb[0:1, 384:768],
                                  in_=tail_qb[0:1, 0:384])
            tqp = psmall()
            for t in range(6):
                nc.tensor.transpose(
                    tqp[:, t:t + 1],
                    tail_qk_sb[0:1, 128 * t:128 * (t + 1)].bitcast(F32),
                    identf[0:1, 0:1])
            tq_sb = nrm_pool.tile([128, 6], BF, tag=f"tqsb{g}")
            nc.vector.tensor_copy(out=tq_sb[:], in_=tqp[:, 0:6])
            yield

            # RoPE on tail column (all 6 tiles at once)
            rp = psmall()
            nc.tensor.matmul(
                rp[:, 0:6],
                lhsT=rot[:],
                rhs=tq_sb[:],
                start=True, stop=True,
            )
            tt1 = nrm_pool.tile([128, 6], F32, tag=f"tail1{g}")
            nc.vector.tensor_tensor(
                tt1[:], rp[:, 0:6],
                sinT[:, 1024:1025].to_broadcast([128, 6]),
                mybir.AluOpType.mult)
            tt2 = nrm_pool.tile([128, 6], F32, tag=f"tail2{g}")
            nc.vector.tensor_tensor(
                tt2[:], tq_sb[:],
                cosT[:, 1024:1025].to_broadcast([128, 6, 1]),
                mybir.AluOpType.mult)
            nc.vector.tensor_tensor(
                qkT[:, 0:6, 1024:1025],
                tt1[:].rearrange("p (a b) -> p a b", b=1),
                tt2[:].rearrange("p (a b) -> p a b", b=1),
                mybir.AluOpType.add)

            nc.vector.tensor_copy(
                out=qkT[:, :, 1025:NPAD],
                in_=zerof[:, 0:1].to_broadcast([128, 6, NPAD - 1025]))
            yield

        def gen_proj(b, attn_outT):
            """Output projection for batch b. Accumulates in psmall pairs."""
            for it in range(NT + 1):
                ydst_rows = 128 if it < NT else 1
                yp_a = psmall()
                yp_b = psmall()
                for ct in range(6):
                    nc.tensor.matmul(
                        yp_a[0:ydst_rows, 0:512],
                        lhsT=attn_outT[:, ct, 128 * it:128 * it + ydst_rows],
                        rhs=wproj[:, ct, 0:512],
                        start=(ct == 0), stop=(ct == 5),
                    )
                    nc.tensor.matmul(
                        yp_b[0:ydst_rows, 0:256],
                        lhsT=attn_outT[:, ct, 128 * it:128 * it + ydst_rows],
                        rhs=wproj[:, ct, 512:768],
                        start=(ct == 0), stop=(ct == 5),
                    )
                    if ct in (1, 3):
                        yield
                ysb = y_pool.tile([128, C], F32, tag="ysb")
                nc.vector.tensor_tensor(
                    ysb[0:ydst_rows, 0:512], yp_a[0:ydst_rows, 0:512],
                    bias_bc[0:ydst_rows, 0:512], mybir.AluOpType.add)
                nc.vector.tensor_tensor(
                    ysb[0:ydst_rows, 512:768], yp_b[0:ydst_rows, 0:256],
                    bias_bc[0:ydst_rows, 512:768], mybir.AluOpType.add)
                nc.sync.dma_start(
                    y[b, 128 * it:128 * it + ydst_rows, :], ysb[0:ydst_rows, :])
                yield

        def emit_attn_head(g, hh, qkT, v_aug, attn_outT, pump):
            pair, half = hh // 2, hh % 2
            r0 = 64 * half
            qh = qkT[r0:r0 + 64, pair, :]
            kh = qkT[r0:r0 + 64, 3 + pair, :]
            vh_t = lambda t: v_aug[:, t, hh, :]

            o_ps = po()
            pts = [None] * (NT + 1)

            def emit_scores(jt):
                sp = ps()
                for c0 in (0, 512):
                    nc.tensor.matmul(
                        sp[:, c0:c0 + 512],
                        lhsT=kh[:, 128 * jt:128 * (jt + 1)],
                        rhs=qh[:, c0:c0 + 512],
                        start=True, stop=True,
                    )
                ptile = pt_pool.tile([128, 1024], BF, tag="pt")
                pts[jt] = ptile
                nc.scalar.activation(ptile[:], sp[:, 0:1024], AF.Exp,
                                     scale=SCALE)

            def emit_pv(jt):
                for c0 in (0, 512):
                    nc.tensor.matmul(
                        o_ps[0:DH + 1, c0:c0 + 512],
                        lhsT=vh_t(jt),
                        rhs=pts[jt][:, c0:c0 + 512],
                        start=(jt == 0), stop=(jt == NT),
                    )
                pts[jt] = None

            emit_scores(0)
            for jt in range(1, NT + 1):
                pump(1)
                emit_scores(jt)
                emit_pv(jt - 1)
                pump(1)
            pump(1)
            emit_pv(NT)

            # stripe B: query token 1024 (sb/ob live in the S rotation)
            sb = ps()
            for jt in range(NT + 1):
                nc.tensor.matmul(
                    sb[:, jt:jt + 1],
                    lhsT=kh[:, 128 * jt:128 * (jt + 1)],
                    rhs=qh[:, 1024:1025],
                    start=True, stop=True,
                )
            ptb = nrm_pool.tile([128, 16], BF, tag="ptb")
            nc.scalar.activation(ptb[:, 0:NT + 1], sb[:, 0:NT + 1],
                                 AF.Exp, scale=SCALE)
            pump(1)
            ob = ps()
            for jt in range(NT + 1):
                nc.tensor.matmul(
                    ob[0:DH + 1, 0:1],
                    lhsT=vh_t(jt),
                    rhs=ptb[:, jt:jt + 1],
                    start=(jt == 0), stop=(jt == NT),
                )

            pump(1)
            # normalize: attn_outT rows = O'/Z
            h_glob = 6 * g + hh
            drow = 64 * (h_glob % 2)
            dtile = h_glob // 2
            # tail query first (frees ob for the next head's S rotation)
            rzb = nrm_pool.tile([1, 16], F32R, tag="rzb")
            nc.scalar.activation(rzb[0:1, 0:1], ob[DH:DH + 1, 0:1], AF.Ln)
            nc.scalar.activation(rzb[0:1, 0:1], rzb[0:1, 0:1], AF.Exp,
                                 scale=-1.0)
            nc.tensor.matmul(
                ob[64:128, 0:1],
                lhsT=ones64[:].bitcast(F32),
                rhs=rzb[0:1, 0:1].bitcast(F32),
                start=True, stop=True,
            )
            rzbbc = nrm_pool.tile([64, 16], F32, tag="rzbbc")
            nc.vector.tensor_copy(out=rzbbc[:, 0:1], in_=ob[64:128, 0:1])
            nc.vector.tensor_tensor(
                attn_outT[drow:drow + 64, dtile, 1024:1025],
                ob[0:DH, 0:1], rzbbc[:, 0:1], mybir.AluOpType.mult)

            # main queries: Ln, then copy O' out unnormalized (frees o_ps
            # early), broadcast 1/Z into an S-rotation tile, scale in place.
            rz = nrm_pool.tile([1, 1024], BF, tag="rz")
            nc.scalar.activation(rz[:], o_ps[DH:DH + 1, 0:1024], AF.Ln)
            aotmp = nrm_pool.tile([64, 1024], F32, tag="aotmp")
            nc.vector.tensor_copy(out=aotmp[:], in_=o_ps[0:DH, 0:1024])
            nc.scalar.activation(rz[:], rz[:], AF.Exp, scale=-1.0)
            bc = ps()
            for c0 in (0, 512):
                nc.tensor.matmul(
                    bc[0:64, c0:c0 + 512],
                    lhsT=ones64b[:],
                    rhs=rz[:, c0:c0 + 512],
                    start=True, stop=True,
                )
            rzbc = nrm_pool.tile([64, 1024], BF, tag="rzbc")
            nc.vector.tensor_copy(out=rzbc[:], in_=bc[0:64, 0:1024])
            nc.vector.tensor_tensor(
                attn_outT[drow:drow + 64, dtile, 0:1024],
                aotmp[:], rzbc[:], mybir.AluOpType.mult)
            pump(1)

        # ---------------- pipelined schedule ----------------
        _SENT = object()

        def make_pump(*gens):
            import itertools
            it = itertools.chain(*gens)

            def pump(n=1):
                for _ in range(n):
                    if next(it, _SENT) is _SENT:
                        return

            def drain():
                for _ in it:
                    pass
            return pump, drain

        def nopump(n=1):
            pass

        def alloc_group(parity):
            qkT = big.tile([128, 6, NPAD], BF, tag=f"qkT{parity}")
            v_aug = big.tile([128, NT + 1, 6, DH + 1], BF, tag=f"v_aug{parity}")
            return qkT, v_aug

        # stage A: xT(0) + qkv(0,0), serial (pipeline fill)
        xT0 = big.tile([128, 6, N], BF, tag="xT0")
        for _ in gen_xT(0, xT0):
            pass
        qkT00, vaug00 = alloc_group(0)
        for _ in gen_qkv(0, 0, xT0, qkT00, vaug00):
            pass

        # stage B: attn(0,0) ⊗ qkv(0,1)
        attn_outT0 = big.tile([128, 6, N], BF, tag="attn_outT0")
        qkT01, vaug01 = alloc_group(1)
        pump, drain = make_pump(gen_qkv(0, 1, xT0, qkT01, vaug01))
        for hh in range(6):
            emit_attn_head(0, hh, qkT00, vaug00, attn_outT0, pump)
        drain()

        # stage C: attn(0,1) ⊗ [xT(1), qkv(1,0)]
        xT1 = big.tile([128, 6, N], BF, tag="xT1")
        qkT10, vaug10 = alloc_group(0)
        pump, drain = make_pump(gen_xT(1, xT1),
                                gen_qkv(1, 0, xT1, qkT10, vaug10))
        for hh in range(6):
            emit_attn_head(1, hh, qkT01, vaug01, attn_outT0, pump)
        drain()

        # stage D: attn(1,0) ⊗ [qkv(1,1), then proj(0)]. qkv(1,1) must fully
        # drain before stage E (its consumer); proj(0) may spill into E.
        attn_outT1 = big.tile([128, 6, N], BF, tag="attn_outT1")
        qkT11, vaug11 = alloc_group(1)
        it_qkv11 = gen_qkv(1, 1, xT1, qkT11, vaug11)
        it_proj0 = gen_proj(0, attn_outT0)

        def pump_d(n=1):
            for _ in range(n):
                if next(it_qkv11, _SENT) is _SENT:
                    if next(it_proj0, _SENT) is _SENT:
                        return

        for hh in range(6):
            emit_attn_head(0, hh, qkT10, vaug10, attn_outT1, pump_d)
        for _ in it_qkv11:
            pass

        # stage E: attn(1,1) ⊗ [rest of proj(0)]
        def pump_e(n=1):
            for _ in range(n):
                if next(it_proj0, _SENT) is _SENT:
                    return

        for hh in range(6):
            emit_attn_head(1, hh, qkT11, vaug11, attn_outT1, pump_e)
        for _ in it_proj0:
            pass

        # stage F: proj(1) serial
        for _ in gen_proj(1, attn_outT1):
            pass

    split_multi_waits(nc)
    return nc


_CACHED = {}


def kernel(**inputs) -> np.ndarray:
    from concourse.bass_utils import run_bass_kernel_spmd

    x = np.ascontiguousarray(np.asarray(inputs["x"], dtype=np.float32))
    B = x.shape[0]
    n_cores = 8
    per = B // n_cores
    if "nc" not in _CACHED:
        _CACHED["nc"] = build_kernel()
    nc = _CACHED["nc"]
    in_maps = []
    for c in range(n_cores):
        in_maps.append({
            "x": np.ascontiguousarray(x[c * per:(c + 1) * per]),
            "sin": np.ascontiguousarray(np.asarray(inputs["sin"], np.float32)),
            "cos": np.ascontiguousarray(np.asarray(inputs["cos"], np.float32)),
            "w_qkv": np.ascontiguousarray(np.asarray(inputs["W_qkv"], np.float32)),
            "w_proj": np.ascontiguousarray(np.asarray(inputs["W_proj"], np.float32)),
            "b_proj": np.ascontiguousarray(np.asarray(inputs["b_proj"], np.float32)),
        })
    res = run_bass_kernel_spmd(nc, in_maps, core_ids=list(range(n_cores)))
    return np.concatenate([res.results[c]["y"] for c in range(n_cores)], axis=0)


# revision 42
# speedup vs baseline: 1.0276x; 1.0276x over previous
"""Trainium2 Bass kernel for nn_Attention_33200097198117.

B=16, N=1025, C=768, H=12 RoPE attention. Data-parallel over batch:
each of the 8 NeuronCores computes 2 batches with the full weights; the
full output is the concatenation over cores (no collectives needed).

kernel(**inputs) -> np.ndarray: builds the Bass/Tile program (cached),
shards inputs, runs on cores 0-7 via bass_utils.run_bass_kernel_spmd,
and concatenates the per-core outputs.
"""

import numpy as np

# ---------------------------------------------------------------------------
# Toolchain compatibility: this container's walrus accepts at most ONE sync
# wait entry per instruction, while Tile's scheduler attaches several (and
# its kernel-tail drain collects one per outstanding semaphore). Patch the
# tail drain and post-process the module to split multi-wait instructions.
# ---------------------------------------------------------------------------
import concourse.tile as tile
from bass_rust import ScopedClock


def _drain_and_barrier(self, tick_clock, wait_clock):
    drain_inst = self.nc.sync.drain()
    wait_clock.add_sem_waits(drain_inst.ins, ScopedClock({None: tick_clock.global_clock}))
    si = drain_inst.ins.sync_info
    waits = list(si.on_wait) if si is not None else []
    if len(waits) > 1:
        si.on_wait = [waits[0]]
        assert self.sems is not None
        allocated = dict(self.sems.allocated())
        by_name = {}
        for v in allocated.values():
            by_name[getattr(v, "name", None)] = v
        for w in waits[1:]:
            sem = by_name.get(w.ant_name) or allocated.get(w.ant_name)
            assert sem is not None, f"sem {w.ant_name} not found"
            nop = self.nc.sync.nop()
            assert w.wait_mode in ("sem-ge-imm", "sem-ge"), w.wait_mode
            nop.wait_op(sem, w.wait_value, "sem-ge")

    self.nc.all_engine_barrier()
    assert self.sems is not None
    popped = self.nc._tile_sem_poison_stack.pop()
    assert popped is self._sem_poison
    self.nc.clear_and_free_semaphores(list(self.sems.allocated().values()))
    self.nc.all_engine_barrier()


tile.TileContext._drain_and_barrier = _drain_and_barrier


def split_multi_waits(nc):
    """Hoist extra sync waits onto cloned NoOps before each instruction."""
    import copy
    import bass_rust

    template = None
    for f in nc.m.functions:
        for b in f.blocks:
            for inst in b.instructions:
                if type(inst).__name__ == "InstNoOp":
                    template = inst
                    break
            if template is not None:
                break
    assert template is not None, "need one InstNoOp in module as clone template"

    for f in nc.m.functions:
        for b in f.blocks:
            changed = False
            out = []
            for inst in b.instructions:
                si = inst.sync_info
                waits = list(si.on_wait) if si is not None else []
                if len(waits) > 1:
                    changed = True
                    for i, w in enumerate(waits[:-1]):
                        n = copy.copy(template)
                        n.name = f"{inst.name}-wsplit{i}"
                        n.engine = inst.engine
                        n.sync_info = bass_rust.SyncInfo(on_wait=[w], on_update=[])
                        out.append(n)
                    si.on_wait = [waits[-1]]
                out.append(inst)
            if changed:
                b.instructions = out


_DOC = """Bass/Tile kernel for nn_Attention (B=16, N=1025, C=768, H=12 RoPE attention).

Sharding: data-parallel over batch. Each of 8 cores processes 2 batches with
full weights; no collectives.

v3: bf16 matmuls, weights resident in SBUF, and software-pipelined emission:
the attention inner loops (ACT-exp-bound) are interleaved chunk-by-chunk with
the NEXT group's QKV projection / transpose / output-projection matmuls so
the PE never idles on softmax waits. Double-buffered xT/qkT/v_aug/attn_outT;
PSUM split into dedicated single-buffer regions per stream (o_ps / S-tile /
qkv accumulator) + 2 shared small banks.

Per-(batch,group) math is identical to v2 (see emit_* functions).
"""

from contextlib import ExitStack

import concourse.bass as bass
import concourse.mybir as mybir
import concourse.tile as tile
from concourse.masks import make_identity

F32 = mybir.dt.float32
F32R = mybir.dt.float32r
BF = mybir.dt.bfloat16
AF = mybir.ActivationFunctionType

B_PER_CORE = 2
N = 1025
C = 768
H = 12
DH = 64
NT = 8          # full 128-token tiles
NPAD = 1152     # qkT free-dim allocation (1024 + 128 zero pad incl. col 1024)
SCALE = DH ** -0.5


def build_rot_matrix(nc, rot):
    """lhsT for rotate_half: out = rot.T @ qT gives rot(q) rows."""
    nc.gpsimd.memset(rot, 0.0)
    for blk in range(2):
        b0 = 64 * blk
        nc.gpsimd.affine_select(
            out=rot[b0:b0 + 32, :],
            in_=rot[b0:b0 + 32, :],
            compare_op=mybir.AluOpType.not_equal,
            fill=1.0,
            base=b0 + 32,
            pattern=[[-1, 128]],
            channel_multiplier=1,
        )
        nc.gpsimd.affine_select(
            out=rot[b0 + 32:b0 + 64, :],
            in_=rot[b0 + 32:b0 + 64, :],
            compare_op=mybir.AluOpType.not_equal,
            fill=-1.0,
            base=b0,
            pattern=[[-1, 128]],
            channel_multiplier=1,
        )


def build_kernel():
    nc = bass.Bass("TRN2", target_bir_lowering=False, debug=False, num_devices=8)

    x = nc.dram_tensor("x", [B_PER_CORE, N, C], F32, kind="ExternalInput").ap()
    sin = nc.dram_tensor("sin", [N - 1, DH], F32, kind="ExternalInput").ap()
    cos = nc.dram_tensor("cos", [N - 1, DH], F32, kind="ExternalInput").ap()
    w_qkv = nc.dram_tensor("w_qkv", [C, 3 * C], F32, kind="ExternalInput").ap()
    w_proj = nc.dram_tensor("w_proj", [C, C], F32, kind="ExternalInput").ap()
    b_proj = nc.dram_tensor("b_proj", [C], F32, kind="ExternalInput").ap()
    y = nc.dram_tensor("y", [B_PER_CORE, N, C], F32, kind="ExternalOutput").ap()

    with tile.TileContext(nc) as tc, ExitStack() as ctx:
        nc.sync.nop(nofuse=True)  # clone template for split_multi_waits
        const = ctx.enter_context(tc.tile_pool(name="const", bufs=1))
        big = ctx.enter_context(tc.tile_pool(name="bigbuf", bufs=1))
        xn_pool = ctx.enter_context(tc.tile_pool(name="xnat", bufs=2))
        pt_pool = ctx.enter_context(tc.tile_pool(name="pt", bufs=3))
        tmp_pool = ctx.enter_context(tc.tile_pool(name="tmp", bufs=2))
        y_pool = ctx.enter_context(tc.tile_pool(name="ystage", bufs=2))
        nrm_pool = ctx.enter_context(tc.tile_pool(name="nrm", bufs=1))
        psum = ctx.enter_context(tc.tile_pool(name="psum", bufs=1, space="PSUM"))
        psum_s = ctx.enter_context(tc.tile_pool(name="psum_s", bufs=2, space="PSUM"))

        # PSUM budget (8 banks): o_ps 2 + S-ping-pong 4 + filler-smalls 2.
        # Attention owns "ops"/"sc"; the interleaved filler stream owns
        # "small" exclusively (so fillers may hold smalls across yields).
        def po():
            return psum.tile([128, 1024], F32, tag="ops", name="po")

        def ps():
            return psum.tile([128, 1024], F32, tag="sc", bufs=2, name="ps")

        def psmall():
            return psum_s.tile([128, 512], F32, tag="small", name="psmall")

        # ---------------- constants ----------------
        identf = const.tile([128, 128], F32, tag="identf")
        make_identity(nc, identf[:])
        ident = const.tile([128, 128], F32R, tag="ident")
        nc.vector.tensor_copy(out=ident[:], in_=identf[:])
        rotf = const.tile([128, 128], F32, tag="rotf")
        build_rot_matrix(nc, rotf[:])
        rot = const.tile([128, 128], BF, tag="rot")
        nc.vector.tensor_copy(out=rot[:], in_=rotf[:])
        onesf = const.tile([128, 1], F32, tag="onesf")
        nc.vector.memset(onesf[:], 1.0)
        zerof = const.tile([128, 1], F32, tag="zerof")
        nc.vector.memset(zerof[:], 0.0)
        ones64 = const.tile([1, 64], F32R, tag="ones64")
        nc.vector.tensor_copy(out=ones64[:], in_=onesf[0:1, 0:1].to_broadcast([1, 64]))
        ones64b = const.tile([1, 64], BF, tag="ones64b")
        nc.vector.tensor_copy(out=ones64b[:], in_=onesf[0:1, 0:1].to_broadcast([1, 64]))

        bias_bc = const.tile([128, C], F32, tag="bias")
        nc.sync.dma_start(bias_bc[0:1, :], b_proj[None, :])
        p = 1
        while p < 128:
            nc.sync.dma_start(bias_bc[p:2 * p, :], bias_bc[0:p, :])
            p *= 2

        # sinT/cosT: [128, N] bf16 coeff col t = (sin,cos) for token t.
        sinT = const.tile([128, N], BF, tag="sinT")
        cosT = const.tile([128, N], BF, tag="cosT")
        nc.vector.memset(sinT[:, 0:1], 0.0)
        nc.vector.memset(cosT[:, 0:1], 1.0)
        sin_nat = sin.rearrange("(o p) d -> p o d", p=128)
        cos_nat = cos.rearrange("(o p) d -> p o d", p=128)
        for src_nat, dstT in ((sin_nat, sinT), (cos_nat, cosT)):
            for t in range(NT):
                nat = tmp_pool.tile([128, DH], F32R, tag="scnat")
                nc.sync.dma_start(nat[:], src_nat[:, t, :].bitcast(F32R))
                pt = psmall()
                nc.tensor.transpose(pt[0:DH, 0:128].bitcast(F32R), nat[:], ident[:])
                nc.vector.tensor_copy(
                    out=dstT[0:DH, 1 + 128 * t:1 + 128 * (t + 1)],
                    in_=pt[0:DH, 0:128],
                )
        nc.sync.dma_start(sinT[64:128, :], sinT[0:64, :])
        nc.sync.dma_start(cosT[64:128, :], cosT[0:64, :])

        # ---------------- resident weights (bf16, loaded once) ----------------
        wq = big.tile([128, 6, 18, 128], BF, tag="wq")
        wdram = w_qkv.rearrange("(k p) (t q) -> p k t q", p=128, q=128)
        for k in range(6):
            for h0 in (0, 9):
                wstage = tmp_pool.tile([128, 9, 128], F32, tag="wstage")
                nc.sync.dma_start(wstage[:], wdram[:, k, h0:h0 + 9, :])
                nc.vector.tensor_copy(out=wq[:, k, h0:h0 + 9, :], in_=wstage[:])
        wproj = big.tile([128, 6, C], BF, tag="wproj")
        for ct in range(6):
            pstage = tmp_pool.tile([128, 9, 128], F32, tag="wstage", name="pstage")
            nc.sync.dma_start(
                pstage[:, 0:6, :],
                w_proj[128 * ct:128 * (ct + 1), :].rearrange("p (a b) -> p a b", b=128))
            nc.vector.tensor_copy(
                out=wproj[:, ct, :],
                in_=pstage[:, 0:6, :].rearrange("p a b -> p (a b)"))

        # ---------------- emission generators ----------------
        def gen_xT(b, xT):
            """Transpose x[b] into xT (bf16). Yields between PSUM chunks."""
            for t in range(NT):
                xnat = xn_pool.tile([128, C], F32R, tag="xnat")
                nc.sync.dma_start(xnat[:], x[b, 128 * t:128 * (t + 1), :].bitcast(F32R))
                for kk in range(0, 6, 4):
                    kw = min(4, 6 - kk)
                    pt = psmall()
                    for j in range(kw):
                        nc.tensor.transpose(
                            pt[:, 128 * j:128 * (j + 1)].bitcast(F32R),
                            xnat[:, 128 * (kk + j):128 * (kk + j + 1)],
                            ident[:],
                        )
                    nc.vector.tensor_copy(
                        out=xT[:, kk:kk + kw, 128 * t:128 * (t + 1)],
                        in_=pt[:, 0:128 * kw].rearrange("p (a b) -> p a b", a=kw),
                    )
                    yield
            xtail = nrm_pool.tile([128, 6, 1], F32, tag=f"xtail{b}")
            with nc.allow_non_contiguous_dma(reason="single tail token scatter"):
                nc.sync.dma_start(
                    xtail[:],
                    x[b, 1024, :].rearrange("(k p a) -> p k a", p=128, a=1),
                )
            nc.vector.tensor_copy(out=xT[:, :, 1024:1025], in_=xtail[:])
            yield

        def gen_qkv(b, g, xT, qkT, v_aug):
            """QKV projection + rope for head group g. Yields ~every 0.5-1us
            of PE work so attention can interleave."""
            nc.vector.tensor_copy(
                out=v_aug[:, :, :, DH:DH + 1].rearrange("p a b c -> p (a b) c"),
                in_=onesf[:, 0:1].to_broadcast([128, (NT + 1) * 6, 1]))

            def wqk(k, m):
                t = 3 * g + m if m < 3 else 6 + 3 * g + (m - 3)
                return wq[:, k, t, :]

            for m in range(6):
                qp_a = psmall()
                qp_b = psmall()
                for k in range(6):
                    nc.tensor.matmul(
                        qp_a[:, 0:512],
                        lhsT=wqk(k, m),
                        rhs=xT[:, k, 0:512],
                        start=(k == 0), stop=(k == 5),
                    )
                    nc.tensor.matmul(
                        qp_b[:, 0:512],
                        lhsT=wqk(k, m),
                        rhs=xT[:, k, 512:1024],
                        start=(k == 0), stop=(k == 5),
                    )
                    yield
                raw = tmp_pool.tile([128, 1024], BF, tag="qkraw")
                nc.vector.tensor_copy(out=raw[:, 0:512], in_=qp_a[:, 0:512])
                nc.vector.tensor_copy(out=raw[:, 512:1024], in_=qp_b[:, 0:512])
                rp_a = psmall()
                rp_b = psmall()
                nc.tensor.matmul(rp_a[:, 0:512], lhsT=rot[:],
                                 rhs=raw[:, 0:512], start=True, stop=True)
                nc.tensor.matmul(rp_b[:, 0:512], lhsT=rot[:],
                                 rhs=raw[:, 512:1024], start=True, stop=True)
                yield
                t1 = tmp_pool.tile([128, 1024], BF, tag="ropet1")
                nc.vector.tensor_tensor(
                    t1[:, 0:512], rp_a[:, 0:512], sinT[:, 0:512],
                    mybir.AluOpType.mult)
                nc.vector.tensor_tensor(
                    t1[:, 512:1024], rp_b[:, 0:512], sinT[:, 512:1024],
                    mybir.AluOpType.mult)
                nc.vector.tensor_tensor(
                    raw[:], raw[:], cosT[:, 0:1024], mybir.AluOpType.mult)
                nc.gpsimd.tensor_tensor(
                    qkT[:, m, 0:1024], t1[:], raw[:], mybir.AluOpType.add)
                yield

            for t in range(NT):
                vp = psmall()
                for k in range(6):
                    nc.tensor.matmul(
                        vp[:, 0:384],
                        lhsT=xT[:, k, 128 * t:128 * (t + 1)],
                        rhs=wq[:, k, 12 + 3 * g:12 + 3 * g + 3, :].rearrange(
                            "p a b -> p (a b)"),
                        start=(k == 0), stop=(k == 5),
                    )
                nc.vector.tensor_copy(
                    out=v_aug[:, t, :, 0:DH],
                    in_=vp[:, 0:384].rearrange("p (a b) -> p a b", a=6),
                )
                yield

            # tail token: row-form qkv. v first (1 psmall), then q/k as a
            # psmall pair — never more than 2 live smalls.
            tail_v = psmall()
            for k in range(6):
                nc.tensor.matmul(
                    tail_v[0:1, 0:384],
                    lhsT=xT[:, k, 1024:1025],
                    rhs=wq[:, k, 12 + 3 * g:12 + 3 * g + 3, :].rearrange(
                        "p a b -> p (a b)"),
                    start=(k == 0), stop=(k == 5),
                )
            nc.vector.tensor_copy(
                out=v_aug[:, NT, :, :],
                in_=zerof[:, 0:1].to_broadcast([128, 6, DH + 1]))
            nc.vector.tensor_copy(
                out=v_aug[0:1, NT, :, 0:DH],
                in_=tail_v[0:1, 0:384].rearrange("p (a b) -> p a b", a=6),
            )
            nc.vector.tensor_copy(
                out=v_aug[0:1, NT, :, DH:DH + 1],
                in_=onesf[0:1, 0:1].to_broadcast([1, 6, 1]))
            yield

            tail_qa = psmall()
            tail_qb = psmall()
            for k in range(6):
                nc.tensor.matmul(
                    tail_qa[0:1, 0:384],
                    lhsT=xT[:, k, 1024:1025],
                    rhs=wq[:, k, 3 * g:3 * g + 3, :].rearrange("p a b -> p (a b)"),
                    start=(k == 0), stop=(k == 5),
                )
                nc.tensor.matmul(
                    tail_qb[0:1, 0:384],
                    lhsT=xT[:, k, 1024:1025],
                    rhs=wq[:, k, 6 + 3 * g:6 + 3 * g + 3, :].rearrange(
                        "p a b -> p (a b)"),
                    start=(k == 0), stop=(k == 5),
                )

            tail_qk_sb = nrm_pool.tile([1, 768], F32R, tag=f"tailqksb{g}")
            nc.vector.tensor_copy(out=tail_qk_sb[0:1, 0:384],
                                  in_=tail_qa[0:1, 0:384])
            nc.vector.tensor_copy(out=tail_qk_sb[0:1, 384:768],
                                  in_=tail_qb[0:1, 0:384])
            tqp = psmall()
            for t in range(6):
                nc.tensor.transpose(
                    tqp[:, t:t + 1],
                    tail_qk_sb[0:1, 128 * t:128 * (t + 1)].bitcast(F32),
                    identf[0:1, 0:1])
            tq_sb = nrm_pool.tile([128, 6], BF, tag=f"tqsb{g}")
            nc.vector.tensor_copy(out=tq_sb[:], in_=tqp[:, 0:6])
            yield

            # RoPE on tail column (all 6 tiles at once)
            rp = psmall()
            nc.tensor.matmul(
                rp[:, 0:6],
                lhsT=rot[:],
                rhs=tq_sb[:],
                start=True, stop=True,
            )
            tt1 = nrm_pool.tile([128, 6], F32, tag=f"tail1{g}")
            nc.vector.tensor_tensor(
                tt1[:], rp[:, 0:6],
                sinT[:, 1024:1025].to_broadcast([128, 6]),
                mybir.AluOpType.mult)
            tt2 = nrm_pool.tile([128, 6], F32, tag=f"tail2{g}")
            nc.vector.tensor_tensor(
                tt2[:], tq_sb[:],
                cosT[:, 1024:1025].to_broadcast([128, 6, 1]),
                mybir.AluOpType.mult)
            nc.vector.tensor_tensor(
                qkT[:, 0:6, 1024:1025],
                tt1[:].rearrange("p (a b) -> p a b", b=1),
                tt2[:].rearrange("p (a b) -> p a b", b=1),
                mybir.AluOpType.add)

            nc.vector.tensor_copy(
                out=qkT[:, :, 1025:NPAD],
                in_=zerof[:, 0:1].to_broadcast([128, 6, NPAD - 1025]))
            yield

        def gen_proj(b, attn_outT):
            """Output projection for batch b. Accumulates in psmall pairs."""
            for it in range(NT + 1):
                ydst_rows = 128 if it < NT else 1
                yp_a = psmall()
                yp_b = psmall()
                for ct in range(6):
                    nc.tensor.matmul(
                        yp_a[0:ydst_rows, 0:512],
                        lhsT=attn_outT[:, ct, 128 * it:128 * it + ydst_rows],
                        rhs=wproj[:, ct, 0:512],
                        start=(ct == 0), stop=(ct == 5),
                    )
                    nc.tensor.matmul(
                        yp_b[0:ydst_rows, 0:256],
                        lhsT=attn_outT[:, ct, 128 * it:128 * it + ydst_rows],
                        rhs=wproj[:, ct, 512:768],
                        start=(ct == 0), stop=(ct == 5),
                    )
                    if ct in (1, 3):
                        yield
                ysb = y_pool.tile([128, C], F32, tag="ysb")
                nc.vector.tensor_tensor(
                    ysb[0:ydst_rows, 0:512], yp_a[0:ydst_rows, 0:512],
                    bias_bc[0:ydst_rows, 0:512], mybir.AluOpType.add)
                nc.vector.tensor_tensor(
                    ysb[0:ydst_rows, 512:768], yp_b[0:ydst_rows, 0:256],
                    bias_bc[0:ydst_rows, 512:768], mybir.AluOpType.add)
                nc.sync.dma_start(
                    y[b, 128 * it:128 * it + ydst_rows, :], ysb[0:ydst_rows, :])
                yield

        def emit_attn_head(g, hh, qkT, v_aug, attn_outT, pump):
            pair, half = hh // 2, hh % 2
            r0 = 64 * half
            qh = qkT[r0:r0 + 64, pair, :]
            kh = qkT[r0:r0 + 64, 3 + pair, :]
            vh_t = lambda t: v_aug[:, t, hh, :]

            o_ps = po()
            pts = [None] * (NT + 1)

            def emit_scores(jt):
                sp = ps()
                for c0 in (0, 512):
                    nc.tensor.matmul(
                        sp[:, c0:c0 + 512],
                        lhsT=kh[:, 128 * jt:128 * (jt + 1)],
                        rhs=qh[:, c0:c0 + 512],
                        start=True, stop=True,
                    )
                ptile = pt_pool.tile([128, 1024], BF, tag="pt")
                pts[jt] = ptile
                # split exp into halves: PV chunk c0 only depends on its own
                # half, so its first matmul starts ~one half-exp earlier
                nc.scalar.activation(ptile[:, 0:512], sp[:, 0:512], AF.Exp,
                                     scale=SCALE)
                nc.scalar.activation(ptile[:, 512:1024], sp[:, 512:1024],
                                     AF.Exp, scale=SCALE)

            def emit_pv(jt):
                for c0 in (0, 512):
                    nc.tensor.matmul(
                        o_ps[0:DH + 1, c0:c0 + 512],
                        lhsT=vh_t(jt),
                        rhs=pts[jt][:, c0:c0 + 512],
                        start=(jt == 0), stop=(jt == NT),
                    )
                pts[jt] = None

            emit_scores(0)
            for jt in range(1, NT + 1):
                pump(1)
                emit_scores(jt)
                emit_pv(jt - 1)
                pump(1)
            pump(1)
            emit_pv(NT)

            # stripe B: query token 1024 (sb/ob live in the S rotation)
            sb = ps()
            for jt in range(NT + 1):
                nc.tensor.matmul(
                    sb[:, jt:jt + 1],
                    lhsT=kh[:, 128 * jt:128 * (jt + 1)],
                    rhs=qh[:, 1024:1025],
                    start=True, stop=True,
                )
            ptb = nrm_pool.tile([128, 16], BF, tag="ptb")
            nc.scalar.activation(ptb[:, 0:NT + 1], sb[:, 0:NT + 1],
                                 AF.Exp, scale=SCALE)
            pump(1)
            ob = ps()
            for jt in range(NT + 1):
                nc.tensor.matmul(
                    ob[0:DH + 1, 0:1],
                    lhsT=vh_t(jt),
                    rhs=ptb[:, jt:jt + 1],
                    start=(jt == 0), stop=(jt == NT),
                )

            pump(1)
            # normalize: attn_outT rows = O'/Z
            h_glob = 6 * g + hh
            drow = 64 * (h_glob % 2)
            dtile = h_glob // 2
            # tail query first (frees ob for the next head's S rotation)
            rzb = nrm_pool.tile([1, 16], F32R, tag="rzb")
            nc.scalar.activation(rzb[0:1, 0:1], ob[DH:DH + 1, 0:1], AF.Ln)
            nc.scalar.activation(rzb[0:1, 0:1], rzb[0:1, 0:1], AF.Exp,
                                 scale=-1.0)
            nc.tensor.matmul(
                ob[64:128, 0:1],
                lhsT=ones64[:].bitcast(F32),
                rhs=rzb[0:1, 0:1].bitcast(F32),
                start=True, stop=True,
            )
            rzbbc = nrm_pool.tile([64, 16], F32, tag="rzbbc")
            nc.vector.tensor_copy(out=rzbbc[:, 0:1], in_=ob[64:128, 0:1])
            nc.vector.tensor_tensor(
                attn_outT[drow:drow + 64, dtile, 1024:1025],
                ob[0:DH, 0:1], rzbbc[:, 0:1], mybir.AluOpType.mult)

            # main queries: Ln, then copy O' out unnormalized (frees o_ps
            # early), broadcast 1/Z into an S-rotation tile, scale in place.
            rz = nrm_pool.tile([1, 1024], BF, tag="rz")
            nc.scalar.activation(rz[:], o_ps[DH:DH + 1, 0:1024], AF.Ln)
            aotmp = nrm_pool.tile([64, 1024], F32, tag="aotmp")
            nc.vector.tensor_copy(out=aotmp[:], in_=o_ps[0:DH, 0:1024])
            nc.scalar.activation(rz[:], rz[:], AF.Exp, scale=-1.0)
            bc = ps()
            for c0 in (0, 512):
                nc.tensor.matmul(
                    bc[0:64, c0:c0 + 512],
                    lhsT=ones64b[:],
                    rhs=rz[:, c0:c0 + 512],
                    start=True, stop=True,
                )
            rzbc = nrm_pool.tile([64, 1024], BF, tag="rzbc")
            nc.vector.tensor_copy(out=rzbc[:], in_=bc[0:64, 0:1024])
            nc.vector.tensor_tensor(
                attn_outT[drow:drow + 64, dtile, 0:1024],
                aotmp[:], rzbc[:], mybir.AluOpType.mult)
            pump(1)

        # ---------------- pipelined schedule ----------------
        _SENT = object()

        def make_pump(*gens):
            import itertools
            it = itertools.chain(*gens)

            def pump(n=1):
                for _ in range(n):
                    if next(it, _SENT) is _SENT:
                        return

            def drain():
                for _ in it:
                    pass
            return pump, drain

        def nopump(n=1):
            pass

        def alloc_group(parity):
            qkT = big.tile([128, 6, NPAD], BF, tag=f"qkT{parity}")
            v_aug = big.tile([128, NT + 1, 6, DH + 1], BF, tag=f"v_aug{parity}")
            return qkT, v_aug

        # stage A: xT(0) + qkv(0,0), serial (pipeline fill)
        xT0 = big.tile([128, 6, N], BF, tag="xT0")
        for _ in gen_xT(0, xT0):
            pass
        qkT00, vaug00 = alloc_group(0)
        for _ in gen_qkv(0, 0, xT0, qkT00, vaug00):
            pass

        # stage B: attn(0,0) ⊗ qkv(0,1)
        attn_outT0 = big.tile([128, 6, N], BF, tag="attn_outT0")
        qkT01, vaug01 = alloc_group(1)
        pump, drain = make_pump(gen_qkv(0, 1, xT0, qkT01, vaug01))
        for hh in range(6):
            emit_attn_head(0, hh, qkT00, vaug00, attn_outT0, pump)
        drain()

        # stage C: attn(0,1) ⊗ [xT(1), qkv(1,0)]
        xT1 = big.tile([128, 6, N], BF, tag="xT1")
        qkT10, vaug10 = alloc_group(0)
        pump, drain = make_pump(gen_xT(1, xT1),
                                gen_qkv(1, 0, xT1, qkT10, vaug10))
        for hh in range(6):
            emit_attn_head(1, hh, qkT01, vaug01, attn_outT0, pump)
        drain()

        # stage D: attn(1,0) ⊗ [qkv(1,1), then proj(0)]. qkv(1,1) must fully
        # drain before stage E (its consumer); proj(0) may spill into E.
        attn_outT1 = big.tile([128, 6, N], BF, tag="attn_outT1")
        qkT11, vaug11 = alloc_group(1)
        it_qkv11 = gen_qkv(1, 1, xT1, qkT11, vaug11)
        it_proj0 = gen_proj(0, attn_outT0)

        def pump_d(n=1):
            for _ in range(n):
                if next(it_qkv11, _SENT) is _SENT:
                    if next(it_proj0, _SENT) is _SENT:
                        return

        for hh in range(6):
            emit_attn_head(0, hh, qkT10, vaug10, attn_outT1, pump_d)
        for _ in it_qkv11:
            pass

        # stage E: attn(1,1) ⊗ [rest of proj(0)]
        def pump_e(n=1):
            for _ in range(n):
                if next(it_proj0, _SENT) is _SENT:
                    return

        for hh in range(6):
            emit_attn_head(1, hh, qkT11, vaug11, attn_outT1, pump_e)
        for _ in it_proj0:
            pass

        # stage F: proj(1) serial
        for _ in gen_proj(1, attn_outT1):
            pass

    split_multi_waits(nc)
    return nc


_CACHED = {}


def kernel(**inputs) -> np.ndarray:
    from concourse.bass_utils import run_bass_kernel_spmd

    x = np.ascontiguousarray(np.asarray(inputs["x"], dtype=np.float32))
    B = x.shape[0]
    n_cores = 8
    per = B // n_cores
    if "nc" not in _CACHED:
        _CACHED["nc"] = build_kernel()
    nc = _CACHED["nc"]
    in_maps = []
    for c in range(n_cores):
        in_maps.append({
            "x": np.ascontiguousarray(x[c * per:(c + 1) * per]),
            "sin": np.ascontiguousarray(np.asarray(inputs["sin"], np.float32)),
            "cos": np.ascontiguousarray(np.asarray(inputs["cos"], np.float32)),
            "w_qkv": np.ascontiguousarray(np.asarray(inputs["W_qkv"], np.float32)),
            "w_proj": np.ascontiguousarray(np.asarray(inputs["W_proj"], np.float32)),
            "b_proj": np.ascontiguousarray(np.asarray(inputs["b_proj"], np.float32)),
        })
    res = run_bass_kernel_spmd(nc, in_maps, core_ids=list(range(n_cores)))
    return np.concatenate([res.results[c]["y"] for c in range(n_cores)], axis=0)


# revision 43
# speedup vs baseline: 1.1739x; 1.1423x over previous
"""Trainium2 Bass kernel for nn_Attention_33200097198117.

B=16, N=1025, C=768, H=12 RoPE attention. Data-parallel over batch:
each of the 8 NeuronCores computes 2 batches with the full weights; the
full output is the concatenation over cores (no collectives needed).

kernel(**inputs) -> np.ndarray: builds the Bass/Tile program (cached),
shards inputs, runs on cores 0-7 via bass_utils.run_bass_kernel_spmd,
and concatenates the per-core outputs.
"""

import numpy as np

# ---------------------------------------------------------------------------
# Toolchain compatibility: this container's walrus accepts at most ONE sync
# wait entry per instruction, while Tile's scheduler attaches several (and
# its kernel-tail drain collects one per outstanding semaphore). Patch the
# tail drain and post-process the module to split multi-wait instructions.
# ---------------------------------------------------------------------------
import concourse.tile as tile
from bass_rust import ScopedClock


def _drain_and_barrier(self, tick_clock, wait_clock):
    drain_inst = self.nc.sync.drain()
    wait_clock.add_sem_waits(drain_inst.ins, ScopedClock({None: tick_clock.global_clock}))
    si = drain_inst.ins.sync_info
    waits = list(si.on_wait) if si is not None else []
    if len(waits) > 1:
        si.on_wait = [waits[0]]
        assert self.sems is not None
        allocated = dict(self.sems.allocated())
        by_name = {}
        for v in allocated.values():
            by_name[getattr(v, "name", None)] = v
        for w in waits[1:]:
            sem = by_name.get(w.ant_name) or allocated.get(w.ant_name)
            assert sem is not None, f"sem {w.ant_name} not found"
            nop = self.nc.sync.nop()
            assert w.wait_mode in ("sem-ge-imm", "sem-ge"), w.wait_mode
            nop.wait_op(sem, w.wait_value, "sem-ge")

    self.nc.all_engine_barrier()
    assert self.sems is not None
    popped = self.nc._tile_sem_poison_stack.pop()
    assert popped is self._sem_poison
    self.nc.clear_and_free_semaphores(list(self.sems.allocated().values()))
    self.nc.all_engine_barrier()


tile.TileContext._drain_and_barrier = _drain_and_barrier


def split_multi_waits(nc):
    """Hoist extra sync waits onto cloned NoOps before each instruction."""
    import copy
    import bass_rust

    template = None
    for f in nc.m.functions:
        for b in f.blocks:
            for inst in b.instructions:
                if type(inst).__name__ == "InstNoOp":
                    template = inst
                    break
            if template is not None:
                break
    assert template is not None, "need one InstNoOp in module as clone template"

    for f in nc.m.functions:
        for b in f.blocks:
            changed = False
            out = []
            for inst in b.instructions:
                si = inst.sync_info
                waits = list(si.on_wait) if si is not None else []
                if len(waits) > 1:
                    changed = True
                    for i, w in enumerate(waits[:-1]):
                        n = copy.copy(template)
                        n.name = f"{inst.name}-wsplit{i}"
                        n.engine = inst.engine
                        n.sync_info = bass_rust.SyncInfo(on_wait=[w], on_update=[])
                        out.append(n)
                    si.on_wait = [waits[-1]]
                out.append(inst)
            if changed:
                b.instructions = out


_DOC = """Bass/Tile kernel for nn_Attention (B=16, N=1025, C=768, H=12 RoPE attention).

Sharding: data-parallel over batch. Each of 8 cores processes 2 batches with
full weights; no collectives.

v3: bf16 matmuls, weights resident in SBUF, and software-pipelined emission:
the attention inner loops (ACT-exp-bound) are interleaved chunk-by-chunk with
the NEXT group's QKV projection / transpose / output-projection matmuls so
the PE never idles on softmax waits. Double-buffered xT/qkT/v_aug/attn_outT;
PSUM split into dedicated single-buffer regions per stream (o_ps / S-tile /
qkv accumulator) + 2 shared small banks.

Per-(batch,group) math is identical to v2 (see emit_* functions).
"""

from contextlib import ExitStack

import concourse.bass as bass
import concourse.mybir as mybir
import concourse.tile as tile
from concourse.masks import make_identity

F32 = mybir.dt.float32
F32R = mybir.dt.float32r
BF = mybir.dt.bfloat16
AF = mybir.ActivationFunctionType

B_PER_CORE = 2
N = 1025
C = 768
H = 12
DH = 64
NT = 8          # full 128-token tiles
NPAD = 1152     # qkT free-dim allocation (1024 + 128 zero pad incl. col 1024)
SCALE = DH ** -0.5


def build_rot_matrix(nc, rot):
    """lhsT for rotate_half: out = rot.T @ qT gives rot(q) rows."""
    nc.gpsimd.memset(rot, 0.0)
    for blk in range(2):
        b0 = 64 * blk
        nc.gpsimd.affine_select(
            out=rot[b0:b0 + 32, :],
            in_=rot[b0:b0 + 32, :],
            compare_op=mybir.AluOpType.not_equal,
            fill=1.0,
            base=b0 + 32,
            pattern=[[-1, 128]],
            channel_multiplier=1,
        )
        nc.gpsimd.affine_select(
            out=rot[b0 + 32:b0 + 64, :],
            in_=rot[b0 + 32:b0 + 64, :],
            compare_op=mybir.AluOpType.not_equal,
            fill=-1.0,
            base=b0,
            pattern=[[-1, 128]],
            channel_multiplier=1,
        )


def build_kernel():
    nc = bass.Bass("TRN2", target_bir_lowering=False, debug=False, num_devices=8)

    x = nc.dram_tensor("x", [B_PER_CORE, N, C], F32, kind="ExternalInput").ap()
    sin = nc.dram_tensor("sin", [N - 1, DH], F32, kind="ExternalInput").ap()
    cos = nc.dram_tensor("cos", [N - 1, DH], F32, kind="ExternalInput").ap()
    w_qkv = nc.dram_tensor("w_qkv", [C, 3 * C], F32, kind="ExternalInput").ap()
    w_proj = nc.dram_tensor("w_proj", [C, C], F32, kind="ExternalInput").ap()
    b_proj = nc.dram_tensor("b_proj", [C], F32, kind="ExternalInput").ap()
    y = nc.dram_tensor("y", [B_PER_CORE, N, C], F32, kind="ExternalOutput").ap()

    with tile.TileContext(nc) as tc, ExitStack() as ctx:
        nc.sync.nop(nofuse=True)  # clone template for split_multi_waits
        const = ctx.enter_context(tc.tile_pool(name="const", bufs=1))
        big = ctx.enter_context(tc.tile_pool(name="bigbuf", bufs=1))
        xn_pool = ctx.enter_context(tc.tile_pool(name="xnat", bufs=2))
        pt_pool = ctx.enter_context(tc.tile_pool(name="pt", bufs=3))
        tmp_pool = ctx.enter_context(tc.tile_pool(name="tmp", bufs=2))
        y_pool = ctx.enter_context(tc.tile_pool(name="ystage", bufs=2))
        nrm_pool = ctx.enter_context(tc.tile_pool(name="nrm", bufs=1))
        psum = ctx.enter_context(tc.tile_pool(name="psum", bufs=1, space="PSUM"))
        psum_s = ctx.enter_context(tc.tile_pool(name="psum_s", bufs=2, space="PSUM"))

        # PSUM budget (8 banks): o_ps 2 + S-ping-pong 4 + filler-smalls 2.
        # Attention owns "ops"/"sc"; the interleaved filler stream owns
        # "small" exclusively (so fillers may hold smalls across yields).
        def po():
            return psum.tile([128, 1024], F32, tag="ops", name="po")

        def ps():
            return psum.tile([128, 1024], F32, tag="sc", bufs=2, name="ps")

        def psmall():
            return psum_s.tile([128, 512], F32, tag="small", name="psmall")

        # ---------------- constants ----------------
        identf = const.tile([128, 128], F32, tag="identf")
        make_identity(nc, identf[:])
        ident = const.tile([128, 128], F32R, tag="ident")
        nc.vector.tensor_copy(out=ident[:], in_=identf[:])
        rotf = const.tile([128, 128], F32, tag="rotf")
        build_rot_matrix(nc, rotf[:])
        rot = const.tile([128, 128], BF, tag="rot")
        nc.vector.tensor_copy(out=rot[:], in_=rotf[:])
        onesf = const.tile([128, 1], F32, tag="onesf")
        nc.vector.memset(onesf[:], 1.0)
        zerof = const.tile([128, 1], F32, tag="zerof")
        nc.vector.memset(zerof[:], 0.0)
        ones64 = const.tile([1, 64], F32R, tag="ones64")
        nc.vector.tensor_copy(out=ones64[:], in_=onesf[0:1, 0:1].to_broadcast([1, 64]))
        ones64b = const.tile([1, 64], BF, tag="ones64b")
        nc.vector.tensor_copy(out=ones64b[:], in_=onesf[0:1, 0:1].to_broadcast([1, 64]))

        bias_bc = const.tile([128, C], F32, tag="bias")
        nc.sync.dma_start(bias_bc[0:1, :], b_proj[None, :])
        p = 1
        while p < 128:
            nc.sync.dma_start(bias_bc[p:2 * p, :], bias_bc[0:p, :])
            p *= 2

        # sinT/cosT: [128, N] bf16 coeff col t = (sin,cos) for token t.
        sinT = const.tile([128, N], BF, tag="sinT")
        cosT = const.tile([128, N], BF, tag="cosT")
        nc.vector.memset(sinT[:, 0:1], 0.0)
        nc.vector.memset(cosT[:, 0:1], 1.0)
        sin_nat = sin.rearrange("(o p) d -> p o d", p=128)
        cos_nat = cos.rearrange("(o p) d -> p o d", p=128)
        for src_nat, dstT in ((sin_nat, sinT), (cos_nat, cosT)):
            for t in range(NT):
                nat = tmp_pool.tile([128, DH], F32R, tag="scnat")
                nc.sync.dma_start(nat[:], src_nat[:, t, :].bitcast(F32R))
                pt = psmall()
                nc.tensor.transpose(pt[0:DH, 0:128].bitcast(F32R), nat[:], ident[:])
                nc.vector.tensor_copy(
                    out=dstT[0:DH, 1 + 128 * t:1 + 128 * (t + 1)],
                    in_=pt[0:DH, 0:128],
                )
        nc.sync.dma_start(sinT[64:128, :], sinT[0:64, :])
        nc.sync.dma_start(cosT[64:128, :], cosT[0:64, :])

        # ---------------- resident weights (bf16, loaded once) ----------------
        wq = big.tile([128, 6, 18, 128], BF, tag="wq")
        wdram = w_qkv.rearrange("(k p) (t q) -> p k t q", p=128, q=128)
        for k in range(6):
            for h0 in (0, 9):
                wstage = tmp_pool.tile([128, 9, 128], F32, tag="wstage")
                nc.sync.dma_start(wstage[:], wdram[:, k, h0:h0 + 9, :])
                nc.vector.tensor_copy(out=wq[:, k, h0:h0 + 9, :], in_=wstage[:])
        wproj = big.tile([128, 6, C], BF, tag="wproj")
        for ct in range(6):
            pstage = tmp_pool.tile([128, 9, 128], F32, tag="wstage", name="pstage")
            nc.sync.dma_start(
                pstage[:, 0:6, :],
                w_proj[128 * ct:128 * (ct + 1), :].rearrange("p (a b) -> p a b", b=128))
            nc.vector.tensor_copy(
                out=wproj[:, ct, :],
                in_=pstage[:, 0:6, :].rearrange("p a b -> p (a b)"))

        # ---------------- emission generators ----------------
        def gen_xT(b, xT):
            """Transpose x[b] into xT (bf16). Yields between PSUM chunks."""
            for t in range(NT):
                xnat = xn_pool.tile([128, C], F32R, tag="xnat")
                nc.sync.dma_start(xnat[:], x[b, 128 * t:128 * (t + 1), :].bitcast(F32R))
                for kk in range(0, 6, 4):
                    kw = min(4, 6 - kk)
                    pt = psmall()
                    for j in range(kw):
                        nc.tensor.transpose(
                            pt[:, 128 * j:128 * (j + 1)].bitcast(F32R),
                            xnat[:, 128 * (kk + j):128 * (kk + j + 1)],
                            ident[:],
                        )
                    nc.vector.tensor_copy(
                        out=xT[:, kk:kk + kw, 128 * t:128 * (t + 1)],
                        in_=pt[:, 0:128 * kw].rearrange("p (a b) -> p a b", a=kw),
                    )
                    yield
            xtail = nrm_pool.tile([128, 6, 1], F32, tag=f"xtail{b}")
            with nc.allow_non_contiguous_dma(reason="single tail token scatter"):
                nc.sync.dma_start(
                    xtail[:],
                    x[b, 1024, :].rearrange("(k p a) -> p k a", p=128, a=1),
                )
            nc.vector.tensor_copy(out=xT[:, :, 1024:1025], in_=xtail[:])
            yield

        def gen_qkv(b, g, xT, qkT, v_aug, ms=(0, 1, 2, 3, 4, 5),
                    do_rest=True):
            """QKV projection + rope for head group g. Yields ~every 0.5-1us
            of PE work so attention can interleave. `ms` selects which q/k
            m-tiles to emit; `do_rest` emits v/tail/pad (independent of ms)."""
            if do_rest:
                nc.vector.tensor_copy(
                    out=v_aug[:, :, :, DH:DH + 1].rearrange("p a b c -> p (a b) c"),
                    in_=onesf[:, 0:1].to_broadcast([128, (NT + 1) * 6, 1]))

            def wqk(k, m):
                t = 3 * g + m if m < 3 else 6 + 3 * g + (m - 3)
                return wq[:, k, t, :]

            for m in ms:
                qp_a = psmall()
                qp_b = psmall()
                for k in range(6):
                    nc.tensor.matmul(
                        qp_a[:, 0:512],
                        lhsT=wqk(k, m),
                        rhs=xT[:, k, 0:512],
                        start=(k == 0), stop=(k == 5),
                    )
                    nc.tensor.matmul(
                        qp_b[:, 0:512],
                        lhsT=wqk(k, m),
                        rhs=xT[:, k, 512:1024],
                        start=(k == 0), stop=(k == 5),
                    )
                    yield
                raw = tmp_pool.tile([128, 1024], BF, tag="qkraw")
                nc.vector.tensor_copy(out=raw[:, 0:512], in_=qp_a[:, 0:512])
                nc.vector.tensor_copy(out=raw[:, 512:1024], in_=qp_b[:, 0:512])
                rp_a = psmall()
                rp_b = psmall()
                nc.tensor.matmul(rp_a[:, 0:512], lhsT=rot[:],
                                 rhs=raw[:, 0:512], start=True, stop=True)
                nc.tensor.matmul(rp_b[:, 0:512], lhsT=rot[:],
                                 rhs=raw[:, 512:1024], start=True, stop=True)
                yield
                t1 = tmp_pool.tile([128, 1024], BF, tag="ropet1")
                nc.vector.tensor_tensor(
                    t1[:, 0:512], rp_a[:, 0:512], sinT[:, 0:512],
                    mybir.AluOpType.mult)
                nc.vector.tensor_tensor(
                    t1[:, 512:1024], rp_b[:, 0:512], sinT[:, 512:1024],
                    mybir.AluOpType.mult)
                nc.vector.tensor_tensor(
                    raw[:], raw[:], cosT[:, 0:1024], mybir.AluOpType.mult)
                nc.gpsimd.tensor_tensor(
                    qkT[:, m, 0:1024], t1[:], raw[:], mybir.AluOpType.add)
                yield

            if not do_rest:
                return
            for t in range(NT):
                vp = psmall()
                for k in range(6):
                    nc.tensor.matmul(
                        vp[:, 0:384],
                        lhsT=xT[:, k, 128 * t:128 * (t + 1)],
                        rhs=wq[:, k, 12 + 3 * g:12 + 3 * g + 3, :].rearrange(
                            "p a b -> p (a b)"),
                        start=(k == 0), stop=(k == 5),
                    )
                nc.vector.tensor_copy(
                    out=v_aug[:, t, :, 0:DH],
                    in_=vp[:, 0:384].rearrange("p (a b) -> p a b", a=6),
                )
                yield

            # tail token: row-form qkv. v first (1 psmall), then q/k as a
            # psmall pair — never more than 2 live smalls.
            tail_v = psmall()
            for k in range(6):
                nc.tensor.matmul(
                    tail_v[0:1, 0:384],
                    lhsT=xT[:, k, 1024:1025],
                    rhs=wq[:, k, 12 + 3 * g:12 + 3 * g + 3, :].rearrange(
                        "p a b -> p (a b)"),
                    start=(k == 0), stop=(k == 5),
                )
            nc.vector.tensor_copy(
                out=v_aug[:, NT, :, :],
                in_=zerof[:, 0:1].to_broadcast([128, 6, DH + 1]))
            nc.vector.tensor_copy(
                out=v_aug[0:1, NT, :, 0:DH],
                in_=tail_v[0:1, 0:384].rearrange("p (a b) -> p a b", a=6),
            )
            nc.vector.tensor_copy(
                out=v_aug[0:1, NT, :, DH:DH + 1],
                in_=onesf[0:1, 0:1].to_broadcast([1, 6, 1]))
            yield

            tail_qa = psmall()
            tail_qb = psmall()
            for k in range(6):
                nc.tensor.matmul(
                    tail_qa[0:1, 0:384],
                    lhsT=xT[:, k, 1024:1025],
                    rhs=wq[:, k, 3 * g:3 * g + 3, :].rearrange("p a b -> p (a b)"),
                    start=(k == 0), stop=(k == 5),
                )
                nc.tensor.matmul(
                    tail_qb[0:1, 0:384],
                    lhsT=xT[:, k, 1024:1025],
                    rhs=wq[:, k, 6 + 3 * g:6 + 3 * g + 3, :].rearrange(
                        "p a b -> p (a b)"),
                    start=(k == 0), stop=(k == 5),
                )

            tail_qk_sb = nrm_pool.tile([1, 768], F32R, tag=f"tailqksb{g}")
            nc.vector.tensor_copy(out=tail_qk_sb[0:1, 0:384],
                                  in_=tail_qa[0:1, 0:384])
            nc.vector.tensor_copy(out=tail_qk_sb[0:1, 384:768],
                                  in_=tail_qb[0:1, 0:384])
            tqp = psmall()
            for t in range(6):
                nc.tensor.transpose(
                    tqp[:, t:t + 1],
                    tail_qk_sb[0:1, 128 * t:128 * (t + 1)].bitcast(F32),
                    identf[0:1, 0:1])
            tq_sb = nrm_pool.tile([128, 6], BF, tag=f"tqsb{g}")
            nc.vector.tensor_copy(out=tq_sb[:], in_=tqp[:, 0:6])
            yield

            # RoPE on tail column (all 6 tiles at once)
            rp = psmall()
            nc.tensor.matmul(
                rp[:, 0:6],
                lhsT=rot[:],
                rhs=tq_sb[:],
                start=True, stop=True,
            )
            tt1 = nrm_pool.tile([128, 6], F32, tag=f"tail1{g}")
            nc.vector.tensor_tensor(
                tt1[:], rp[:, 0:6],
                sinT[:, 1024:1025].to_broadcast([128, 6]),
                mybir.AluOpType.mult)
            tt2 = nrm_pool.tile([128, 6], F32, tag=f"tail2{g}")
            nc.vector.tensor_tensor(
                tt2[:], tq_sb[:],
                cosT[:, 1024:1025].to_broadcast([128, 6, 1]),
                mybir.AluOpType.mult)
            nc.vector.tensor_tensor(
                qkT[:, 0:6, 1024:1025],
                tt1[:].rearrange("p (a b) -> p a b", b=1),
                tt2[:].rearrange("p (a b) -> p a b", b=1),
                mybir.AluOpType.add)

            nc.vector.tensor_copy(
                out=qkT[:, :, 1025:NPAD],
                in_=zerof[:, 0:1].to_broadcast([128, 6, NPAD - 1025]))
            yield

        def gen_proj(b, attn_outT):
            """Output projection for batch b. Accumulates in psmall pairs."""
            for it in range(NT + 1):
                ydst_rows = 128 if it < NT else 1
                yp_a = psmall()
                yp_b = psmall()
                for ct in range(6):
                    nc.tensor.matmul(
                        yp_a[0:ydst_rows, 0:512],
                        lhsT=attn_outT[:, ct, 128 * it:128 * it + ydst_rows],
                        rhs=wproj[:, ct, 0:512],
                        start=(ct == 0), stop=(ct == 5),
                    )
                    nc.tensor.matmul(
                        yp_b[0:ydst_rows, 0:256],
                        lhsT=attn_outT[:, ct, 128 * it:128 * it + ydst_rows],
                        rhs=wproj[:, ct, 512:768],
                        start=(ct == 0), stop=(ct == 5),
                    )
                    if ct in (1, 3):
                        yield
                ysb = y_pool.tile([128, C], F32, tag="ysb")
                nc.vector.tensor_tensor(
                    ysb[0:ydst_rows, 0:512], yp_a[0:ydst_rows, 0:512],
                    bias_bc[0:ydst_rows, 0:512], mybir.AluOpType.add)
                nc.vector.tensor_tensor(
                    ysb[0:ydst_rows, 512:768], yp_b[0:ydst_rows, 0:256],
                    bias_bc[0:ydst_rows, 512:768], mybir.AluOpType.add)
                nc.sync.dma_start(
                    y[b, 128 * it:128 * it + ydst_rows, :], ysb[0:ydst_rows, :])
                yield

        def emit_attn_head(g, hh, qkT, v_aug, attn_outT, pump):
            pair, half = hh // 2, hh % 2
            r0 = 64 * half
            qh = qkT[r0:r0 + 64, pair, :]
            kh = qkT[r0:r0 + 64, 3 + pair, :]
            vh_t = lambda t: v_aug[:, t, hh, :]

            o_ps = po()
            pts = [None] * (NT + 1)

            def emit_scores(jt):
                sp = ps()
                for c0 in (0, 512):
                    nc.tensor.matmul(
                        sp[:, c0:c0 + 512],
                        lhsT=kh[:, 128 * jt:128 * (jt + 1)],
                        rhs=qh[:, c0:c0 + 512],
                        start=True, stop=True,
                    )
                ptile = pt_pool.tile([128, 1024], BF, tag="pt")
                pts[jt] = ptile
                # split exp into halves: PV chunk c0 only depends on its own
                # half, so its first matmul starts ~one half-exp earlier
                nc.scalar.activation(ptile[:, 0:512], sp[:, 0:512], AF.Exp,
                                     scale=SCALE)
                nc.scalar.activation(ptile[:, 512:1024], sp[:, 512:1024],
                                     AF.Exp, scale=SCALE)

            def emit_pv(jt):
                for c0 in (0, 512):
                    nc.tensor.matmul(
                        o_ps[0:DH + 1, c0:c0 + 512],
                        lhsT=vh_t(jt),
                        rhs=pts[jt][:, c0:c0 + 512],
                        start=(jt == 0), stop=(jt == NT),
                    )
                pts[jt] = None

            emit_scores(0)
            for jt in range(1, NT + 1):
                pump(1)
                emit_scores(jt)
                emit_pv(jt - 1)
                pump(1)
            pump(1)
            emit_pv(NT)

            # stripe B: query token 1024 (sb/ob live in the S rotation)
            sb = ps()
            for jt in range(NT + 1):
                nc.tensor.matmul(
                    sb[:, jt:jt + 1],
                    lhsT=kh[:, 128 * jt:128 * (jt + 1)],
                    rhs=qh[:, 1024:1025],
                    start=True, stop=True,
                )
            ptb = nrm_pool.tile([128, 16], BF, tag="ptb")
            nc.scalar.activation(ptb[:, 0:NT + 1], sb[:, 0:NT + 1],
                                 AF.Exp, scale=SCALE)
            pump(1)
            ob = ps()
            for jt in range(NT + 1):
                nc.tensor.matmul(
                    ob[0:DH + 1, 0:1],
                    lhsT=vh_t(jt),
                    rhs=ptb[:, jt:jt + 1],
                    start=(jt == 0), stop=(jt == NT),
                )

            pump(1)
            # normalize: attn_outT rows = O'/Z
            h_glob = 6 * g + hh
            drow = 64 * (h_glob % 2)
            dtile = h_glob // 2
            # tail query first (frees ob for the next head's S rotation)
            rzb = nrm_pool.tile([1, 16], F32R, tag="rzb")
            nc.scalar.activation(rzb[0:1, 0:1], ob[DH:DH + 1, 0:1], AF.Ln)
            nc.scalar.activation(rzb[0:1, 0:1], rzb[0:1, 0:1], AF.Exp,
                                 scale=-1.0)
            nc.tensor.matmul(
                ob[64:128, 0:1],
                lhsT=ones64[:].bitcast(F32),
                rhs=rzb[0:1, 0:1].bitcast(F32),
                start=True, stop=True,
            )
            rzbbc = nrm_pool.tile([64, 16], F32, tag="rzbbc")
            nc.vector.tensor_copy(out=rzbbc[:, 0:1], in_=ob[64:128, 0:1])
            nc.vector.tensor_tensor(
                attn_outT[drow:drow + 64, dtile, 1024:1025],
                ob[0:DH, 0:1], rzbbc[:, 0:1], mybir.AluOpType.mult)

            # main queries: Ln, then copy O' out unnormalized (frees o_ps
            # early), broadcast 1/Z into an S-rotation tile, scale in place.
            rz = nrm_pool.tile([1, 1024], BF, tag="rz")
            nc.scalar.activation(rz[:], o_ps[DH:DH + 1, 0:1024], AF.Ln)
            aotmp = nrm_pool.tile([64, 1024], F32, tag="aotmp")
            nc.vector.tensor_copy(out=aotmp[:], in_=o_ps[0:DH, 0:1024])
            nc.scalar.activation(rz[:], rz[:], AF.Exp, scale=-1.0)
            bc = ps()
            for c0 in (0, 512):
                nc.tensor.matmul(
                    bc[0:64, c0:c0 + 512],
                    lhsT=ones64b[:],
                    rhs=rz[:, c0:c0 + 512],
                    start=True, stop=True,
                )
            rzbc = nrm_pool.tile([64, 1024], BF, tag="rzbc")
            nc.vector.tensor_copy(out=rzbc[:], in_=bc[0:64, 0:1024])
            nc.vector.tensor_tensor(
                attn_outT[drow:drow + 64, dtile, 0:1024],
                aotmp[:], rzbc[:], mybir.AluOpType.mult)
            pump(1)

        # ---------------- pipelined schedule ----------------
        _SENT = object()

        def make_pump(*gens):
            import itertools
            it = itertools.chain(*gens)

            def pump(n=1):
                for _ in range(n):
                    if next(it, _SENT) is _SENT:
                        return

            def drain():
                for _ in it:
                    pass
            return pump, drain

        def nopump(n=1):
            pass

        def alloc_group(parity):
            qkT = big.tile([128, 6, NPAD], BF, tag=f"qkT{parity}")
            v_aug = big.tile([128, NT + 1, 6, DH + 1], BF, tag=f"v_aug{parity}")
            return qkT, v_aug

        # stage A: xT(0) + qkv(0,0), serial (pipeline fill)
        xT0 = big.tile([128, 6, N], BF, tag="xT0")
        for _ in gen_xT(0, xT0):
            pass
        qkT00, vaug00 = alloc_group(0)
        # pipeline fill: only pair-0 q/k tiles + v/tail/pad serially; the
        # remaining m-tiles become stage-B fillers ahead of qkv(0,1)
        for _ in gen_qkv(0, 0, xT0, qkT00, vaug00, ms=(0, 3)):
            pass

        # stage B: attn(0,0) ⊗ [rest of qkv(0,0), qkv(0,1)]
        attn_outT0 = big.tile([128, 6, N], BF, tag="attn_outT0")
        qkT01, vaug01 = alloc_group(1)
        pump, drain = make_pump(
            gen_qkv(0, 0, xT0, qkT00, vaug00, ms=(1, 4, 2, 5), do_rest=False),
            gen_qkv(0, 1, xT0, qkT01, vaug01))
        for hh in range(6):
            emit_attn_head(0, hh, qkT00, vaug00, attn_outT0, pump)
        drain()

        # stage C: attn(0,1) ⊗ [xT(1), qkv(1,0)]
        xT1 = big.tile([128, 6, N], BF, tag="xT1")
        qkT10, vaug10 = alloc_group(0)
        pump, drain = make_pump(gen_xT(1, xT1),
                                gen_qkv(1, 0, xT1, qkT10, vaug10))
        for hh in range(6):
            emit_attn_head(1, hh, qkT01, vaug01, attn_outT0, pump)
        drain()

        # stage D: attn(1,0) ⊗ [qkv(1,1), then proj(0)]. qkv(1,1) must fully
        # drain before stage E (its consumer); proj(0) may spill into E.
        attn_outT1 = big.tile([128, 6, N], BF, tag="attn_outT1")
        qkT11, vaug11 = alloc_group(1)
        it_qkv11 = gen_qkv(1, 1, xT1, qkT11, vaug11)
        it_proj0 = gen_proj(0, attn_outT0)

        def pump_d(n=1):
            for _ in range(n):
                if next(it_qkv11, _SENT) is _SENT:
                    if next(it_proj0, _SENT) is _SENT:
                        return

        for hh in range(6):
            emit_attn_head(0, hh, qkT10, vaug10, attn_outT1, pump_d)
        for _ in it_qkv11:
            pass

        # stage E: attn(1,1) ⊗ [rest of proj(0)]
        def pump_e(n=1):
            for _ in range(n):
                if next(it_proj0, _SENT) is _SENT:
                    return

        for hh in range(6):
            emit_attn_head(1, hh, qkT11, vaug11, attn_outT1, pump_e)
        for _ in it_proj0:
            pass

        # stage F: proj(1) serial
        for _ in gen_proj(1, attn_outT1):
            pass

    split_multi_waits(nc)
    return nc


_CACHED = {}


def kernel(**inputs) -> np.ndarray:
    from concourse.bass_utils import run_bass_kernel_spmd

    x = np.ascontiguousarray(np.asarray(inputs["x"], dtype=np.float32))
    B = x.shape[0]
    n_cores = 8
    per = B // n_cores
    if "nc" not in _CACHED:
        _CACHED["nc"] = build_kernel()
    nc = _CACHED["nc"]
    in_maps = []
    for c in range(n_cores):
        in_maps.append({
            "x": np.ascontiguousarray(x[c * per:(c + 1) * per]),
            "sin": np.ascontiguousarray(np.asarray(inputs["sin"], np.float32)),
            "cos": np.ascontiguousarray(np.asarray(inputs["cos"], np.float32)),
            "w_qkv": np.ascontiguousarray(np.asarray(inputs["W_qkv"], np.float32)),
            "w_proj": np.ascontiguousarray(np.asarray(inputs["W_proj"], np.float32)),
            "b_proj": np.ascontiguousarray(np.asarray(inputs["b_proj"], np.float32)),
        })
    res = run_bass_kernel_spmd(nc, in_maps, core_ids=list(range(n_cores)))
    return np.concatenate([res.results[c]["y"] for c in range(n_cores)], axis=0)
